# revision 8
# baseline (speedup 1.0000x reference)
"""Bass/Tile kernel for nn_MultiHeadAttention (B=2, S=2048, D=1024, H=16).

Sharding: 8 cores = 2 (batch) x 4 (head-chunks of 4 heads).
Each core computes, for its batch b and its 4 heads (2 pairs of 2 heads):
  qpT/kpT = (x @ W{q,k} + b)^T   in [dout, token] bf16 layout
  vp      = v @ Wv + bv          in [token, dout] bf16 layout
  scoresT = kp @ qp^T            per head, [k, q] f32 PSUM
  at      = exp(scoresT) bf16, Z[k] via ACT accum_out
  pv[qh]  = sum_kb (vp/Z)^T @ at  accumulated IN PSUM across all kb
  out[p]  = hcT_p^T @ Wo_p        per-pair bf16 partial (host sums 8 partials)

All matmuls bf16 (rel err ~1.25e-2 vs 2e-2 gate, matches numpy sim exactly).
v3: resident raw q/k/v tiles loaded with 2KB-per-partition DMA lines (2x DMA
efficiency vs 512-token chunks), packed wqkv weight tensor, early-exp pass
(scores for kb 0-3 on the first q-half start right after q-proj chunks 0/1 +
k-proj chunk 0), PV(kb-1) emitted between score groups to keep the PE stream
dense (DVFS ramp needs continuous tensor work), per-pair O-proj overlapped
with the other pair's attention.
"""

import sys

sys.path.insert(0, "/opt/trn_rl_repo")

from contextlib import ExitStack

import numpy as np
import ml_dtypes

import concourse.bass as bass
import concourse.mybir as mybir
import concourse.tile as tile
from concourse import bacc
from concourse.bass_utils import run_bass_kernel_spmd

BF16 = mybir.dt.bfloat16
F32 = mybir.dt.float32
AF = mybir.ActivationFunctionType
ALU = mybir.AluOpType

D = 1024
NK = 8  # k-tiles over D
DOUT = 256  # per-core head dims (4 heads)
NPAIR = 2  # pairs of heads (128 dout each)
HD = 64
S = 2048
B = 2
NKB = S // 128  # k-token blocks
NQH = S // 1024  # 1024-wide q halves
NTC = S // 512  # proj token chunks
NTT = S // 128  # token tiles
EARLY = 4  # kb blocks whose qh=0 scores/exp run during the projection head


def build_kernel():
    nc = bacc.Bacc("TRN2", target_bir_lowering=False, debug=False)

    qT = nc.dram_tensor("qT", [D, S], BF16, kind="ExternalInput")
    kT = nc.dram_tensor("kT", [D, S], BF16, kind="ExternalInput")
    vT = nc.dram_tensor("vT", [D, S], BF16, kind="ExternalInput")
    wqkv = nc.dram_tensor("wqkv", [D, 3 * DOUT], BF16, kind="ExternalInput")
    wo = nc.dram_tensor("wo", [DOUT, D], BF16, kind="ExternalInput")
    bq = nc.dram_tensor("bq", [NPAIR, 128, 1], F32, kind="ExternalInput")
    bk = nc.dram_tensor("bk", [NPAIR, 128, 1], F32, kind="ExternalInput")
    bv = nc.dram_tensor("bv", [DOUT], F32, kind="ExternalInput")
    out = nc.dram_tensor("out", [NPAIR, S, D], BF16, kind="ExternalOutput")

    qTv = qT.ap().rearrange("(t p) s -> t p s", p=128)  # [8, 128, S]
    kTv = kT.ap().rearrange("(t p) s -> t p s", p=128)
    vTv = vT.ap().rearrange("(t p) s -> t p s", p=128)
    wqkvv = wqkv.ap().rearrange("(t p) m -> t p m", p=128)  # [8, 128, 768]
    wov = wo.ap().rearrange("(t p) m -> t p m", p=128)  # [2, 128, 1024]
    bqv = bq.ap().rearrange("a p o -> p a o")  # [128, 2, 1]
    bkv = bk.ap().rearrange("a p o -> p a o")
    outv = out.ap().rearrange("a (t p) m -> a t p m", p=128)  # [2,16,128,1024]

    bv_bcast_ap = bass.AP(tensor=bv.ap().tensor, offset=0, ap=[[0, 128], [1, DOUT]])

    with tile.TileContext(nc) as tc, ExitStack() as ctx:
        sb = ctx.enter_context(tc.tile_pool(name="sb", bufs=1))

        # resident raw inputs + weights
        qT_sb = sb.tile([128, NK, S], BF16, tag="qT")
        kT_sb = sb.tile([128, NK, S], BF16, tag="kT")
        vT_sb = sb.tile([128, NK, S], BF16, tag="vT")
        wqkv_sb = sb.tile([128, NK, 3 * DOUT], BF16, tag="wqkv")
        wo_sb = sb.tile([128, NPAIR, D], BF16, tag="wo")
        bq_sb = sb.tile([128, NPAIR, 1], F32, tag="bq")
        bk_sb = sb.tile([128, NPAIR, 1], F32, tag="bk")
        bv_sb = sb.tile([128, DOUT], F32, tag="bv")

        # projection outputs (resident)
        qpT_sb = sb.tile([128, NPAIR, S], BF16, tag="qpT")
        kpT_sb = sb.tile([128, NPAIR, S], BF16, tag="kpT")
        vp_sb = sb.tile([128, NTT, DOUT], BF16, tag="vp")
        hcT_sb = sb.tile([128, NPAIR, S], BF16, tag="hcT")

        psa = ctx.enter_context(tc.tile_pool(name="ps_all", bufs=1, space="PSUM"))
        asb = ctx.enter_context(tc.tile_pool(name="att_sb", bufs=1))
        osb = ctx.enter_context(tc.tile_pool(name="o_sb", bufs=1))

        # ---- DMA schedule: ordered by first consumption ----
        # wqkv[0] + qA first so the very first matmul can start early.
        nc.sync.dma_start(out=wqkv_sb[:, 0, :], in_=wqkvv[0])
        nc.sync.dma_start(out=bq_sb[:], in_=bqv)
        for kk in range(NK):  # q tokens 0..1023
            nc.sync.dma_start(out=qT_sb[:, kk, 0:1024], in_=qTv[kk][:, 0:1024])
        for kk in range(1, NK):
            nc.sync.dma_start(out=wqkv_sb[:, kk, :], in_=wqkvv[kk])
        nc.sync.dma_start(out=bk_sb[:], in_=bkv)
        nc.sync.dma_start(out=bv_sb[:], in_=bv_bcast_ap)
        for kk in range(NK):  # k tokens 0..1023
            nc.sync.dma_start(out=kT_sb[:, kk, 0:1024], in_=kTv[kk][:, 0:1024])
        for kk in range(NK):  # q tokens 1024..2047
            nc.sync.dma_start(out=qT_sb[:, kk, 1024:2048], in_=qTv[kk][:, 1024:2048])
        for kk in range(NK):  # v tokens 0..1023
            nc.sync.dma_start(out=vT_sb[:, kk, 0:1024], in_=vTv[kk][:, 0:1024])
        for kk in range(NK):  # k tokens 1024..2047
            nc.sync.dma_start(out=kT_sb[:, kk, 1024:2048], in_=kTv[kk][:, 1024:2048])
        for kk in range(NK):  # v tokens 1024..2047
            nc.sync.dma_start(out=vT_sb[:, kk, 1024:2048], in_=vTv[kk][:, 1024:2048])
        for t in range(NPAIR):
            nc.sync.dma_start(out=wo_sb[:, t, :], in_=wov[t])

        # ---- emission helpers ----
        def emit_qkproj_pair(XT_sb, woff, b_sb, XPT, tci, p):
            tsl = slice(tci * 512, tci * 512 + 512)
            ps_t = psa.tile([128, 512], F32, tag="big", bufs=2, name=f"pj{p}")
            for kk in range(NK):
                nc.tensor.matmul(
                    ps_t[:],
                    lhsT=wqkv_sb[:, kk, woff + p * 128 : woff + p * 128 + 128],
                    rhs=XT_sb[:, kk, tsl],
                    start=(kk == 0),
                    stop=(kk == NK - 1),
                )
            nc.vector.tensor_scalar_add(XPT[:, p, tsl], ps_t[:], b_sb[:, p, :])

        def emit_vproj(tt):
            psv = psa.tile([128, DOUT], F32, tag="big", bufs=2, name="projv")
            for kk in range(NK):
                nc.tensor.matmul(
                    psv[:],
                    lhsT=vT_sb[:, kk, tt * 128 : tt * 128 + 128],
                    rhs=wqkv_sb[:, kk, 2 * DOUT : 3 * DOUT],
                    start=(kk == 0),
                    stop=(kk == NK - 1),
                )
            nc.vector.scalar_tensor_tensor(
                out=vp_sb[:, tt, :],
                in0=psv[:],
                scalar=1.0,
                in1=bv_sb[:],
                op0=ALU.mult,
                op1=ALU.add,
            )

        def emit_oproj(p, tt, tail=False):
            ost = osb.tile([128, D], BF16, tag="ost", bufs=3, name="ost")
            for dc in range(2):
                pso = psa.tile([128, 512], F32, tag="big", bufs=2, name=f"o{dc}")
                nc.tensor.matmul(
                    pso[:],
                    lhsT=hcT_sb[:, p, tt * 128 : tt * 128 + 128],
                    rhs=wo_sb[:, p, dc * 512 : dc * 512 + 512],
                    start=True,
                    stop=True,
                )
                if tail and dc == 1:
                    nc.scalar.copy(ost[:, dc * 512 : dc * 512 + 512], pso[:])
                else:
                    nc.vector.tensor_copy(ost[:, dc * 512 : dc * 512 + 512], pso[:])
            nc.sync.dma_start(out=outv[p][tt], in_=ost[:])

        def emit_sc_exp(p, kb, h, qh, z4):
            ksl = slice(kb * 128, kb * 128 + 128)
            hsl = slice(h * 64, h * 64 + 64)
            sc = psa.tile([128, 1024], F32, tag="big", bufs=2, name=f"sc{h}{qh}")
            for qq in range(2):
                qsl = slice(qh * 1024 + qq * 512, qh * 1024 + qq * 512 + 512)
                nc.tensor.matmul(
                    sc[:, qq * 512 : qq * 512 + 512],
                    lhsT=kpT_sb[hsl, p, ksl],
                    rhs=qpT_sb[hsl, p, qsl],
                    start=True,
                    stop=True,
                )
            at = asb.tile([128, 1024], BF16, tag="at", bufs=12, name=f"at{h}{qh}")
            nc.scalar.activation(
                out=at[:], in_=sc[:], func=AF.Exp,
                accum_out=z4[:, h, qh : qh + 1],
            )
            return at

        def emit_zchain(p, kb, z4):
            z2 = asb.tile([128, 2], F32, tag="z2", bufs=4, name="z2")
            nc.vector.tensor_add(z2[:], z4[:, :, 0], z4[:, :, 1])
            rz = asb.tile([128, 2], F32, tag="rz", bufs=4, name="rz")
            nc.vector.reciprocal(rz[:], z2[:])
            vhs = asb.tile([128, 2, HD], BF16, tag="vhs", bufs=6, name="vhs")
            for h in range(2):
                nc.vector.tensor_scalar_mul(
                    vhs[:, h, :],
                    vp_sb[:, kb, p * 128 + h * 64 : p * 128 + h * 64 + 64],
                    rz[:, h : h + 1],
                )
            return vhs

        def emit_pv(kb, ats, vhs, pv_tiles):
            for h in range(2):
                for qh in range(NQH):
                    for qq in range(2):
                        nc.tensor.matmul(
                            pv_tiles[qh][
                                h * 64 : h * 64 + 64, qq * 512 : qq * 512 + 512
                            ],
                            lhsT=vhs[:, h, :],
                            rhs=ats[(h, qh)][:, qq * 512 : qq * 512 + 512],
                            start=(kb == 0),
                            stop=(kb == NKB - 1),
                            tile_position=(0, h * 64),
                            skip_group_check=True,
                        )

        # ---- projection head + early exp pass (pair 0, kb<EARLY, qh=0) ----
        for tci in (0, 1):
            for p in range(NPAIR):
                emit_qkproj_pair(qT_sb, 0, bq_sb, qpT_sb, tci, p)
        for p in range(NPAIR):
            emit_qkproj_pair(kT_sb, DOUT, bk_sb, kpT_sb, 0, p)

        at_early = {}
        z4_early = {}
        for kb in range(EARLY):
            z4 = asb.tile([128, 2, NQH], F32, tag="z4e", bufs=EARLY + 1, name="z4e")
            z4_early[kb] = z4
            for h in range(2):
                at_early[(kb, h)] = emit_sc_exp(0, kb, h, 0, z4)
            # q-proj chunks 2/3 ride the early pass's PE slack
            emit_qkproj_pair(qT_sb, 0, bq_sb, qpT_sb, 2 + kb // 2, kb % 2)
        emit_vproj(0)
        emit_vproj(1)

        # ---- attention main loops ----
        for p in range(NPAIR):
            pv_tiles = [
                psa.tile([128, 1024], F32, tag="pv", bufs=2, name=f"pv{qh}")
                for qh in range(NQH)
            ]
            prev = None
            for kb in range(NKB):
                if p == 0 and kb < EARLY:
                    z4 = z4_early[kb]
                    ats = {
                        (0, 0): at_early[(kb, 0)],
                        (1, 0): at_early[(kb, 1)],
                    }
                    ats[(0, 1)] = emit_sc_exp(p, kb, 0, 1, z4)
                    if prev is not None:
                        emit_pv(kb - 1, prev[0], prev[1], pv_tiles)
                    ats[(1, 1)] = emit_sc_exp(p, kb, 1, 1, z4)
                else:
                    z4 = asb.tile(
                        [128, 2, NQH], F32, tag="z4", bufs=6, name="z4"
                    )
                    ats = {}
                    ats[(0, 0)] = emit_sc_exp(p, kb, 0, 0, z4)
                    ats[(0, 1)] = emit_sc_exp(p, kb, 0, 1, z4)
                    if prev is not None:
                        emit_pv(kb - 1, prev[0], prev[1], pv_tiles)
                    ats[(1, 0)] = emit_sc_exp(p, kb, 1, 0, z4)
                    ats[(1, 1)] = emit_sc_exp(p, kb, 1, 1, z4)
                vhs = emit_zchain(p, kb, z4)
                prev = (ats, vhs)
                # trickled, dependency-free PE filler
                if p == 0:
                    if kb < 14:
                        emit_vproj(kb + 2)
                    if kb in (0, 1):
                        emit_qkproj_pair(kT_sb, DOUT, bk_sb, kpT_sb, 1, kb)
                    if kb in (5, 6):
                        emit_qkproj_pair(
                            kT_sb, DOUT, bk_sb, kpT_sb, 2, kb - 5
                        )
                    if kb in (9, 10):
                        emit_qkproj_pair(
                            kT_sb, DOUT, bk_sb, kpT_sb, 3, kb - 9
                        )
                else:
                    emit_oproj(0, kb)
            emit_pv(NKB - 1, prev[0], prev[1], pv_tiles)
            for qh in range(NQH):
                nc.vector.tensor_copy(
                    hcT_sb[:, p, qh * 1024 : qh * 1024 + 1024], pv_tiles[qh][:]
                )

        # ---- pair-1 O-projection tail ----
        for tt in range(NTT):
            emit_oproj(1, tt, tail=True)

    nc.compile()
    return nc


# ---------------- host-side shard / unshard ----------------

_NC_CACHE = {}


def _get_nc():
    if "nc" not in _NC_CACHE:
        _NC_CACHE["nc"] = build_kernel()
    return _NC_CACHE["nc"]


def make_in_maps(q, k, v, Wq, bq, Wk, bk, Wv, bv, Wo, bo):
    bf = ml_dtypes.bfloat16
    maps = []
    qb = [np.ascontiguousarray(q[b].T.astype(bf)) for b in range(B)]
    kb_ = [np.ascontiguousarray(k[b].T.astype(bf)) for b in range(B)]
    vb = [np.ascontiguousarray(v[b].T.astype(bf)) for b in range(B)]
    for c in range(8):
        b = c // 4
        hc = c % 4
        cols = slice(256 * hc, 256 * hc + 256)
        wqkv = np.concatenate(
            [Wq[:, cols], Wk[:, cols], Wv[:, cols]], axis=1
        ).astype(bf)
        maps.append({
            "qT": qb[b],
            "kT": kb_[b],
            "vT": vb[b],
            "wqkv": np.ascontiguousarray(wqkv),
            "wo": np.ascontiguousarray(Wo[cols, :].astype(bf)),
            "bq": np.ascontiguousarray(
                bq[cols].reshape(NPAIR, 128, 1).astype(np.float32)
            ),
            "bk": np.ascontiguousarray(
                bk[cols].reshape(NPAIR, 128, 1).astype(np.float32)
            ),
            "bv": np.ascontiguousarray(bv[cols].astype(np.float32)),
        })
    return maps


def kernel(q, k, v, Wq, bq, Wk, bk, Wv, bv, Wo, bo):
    q = np.asarray(q, dtype=np.float32)
    k = np.asarray(k, dtype=np.float32)
    v = np.asarray(v, dtype=np.float32)
    Wq = np.asarray(Wq, dtype=np.float32)
    Wk = np.asarray(Wk, dtype=np.float32)
    Wv = np.asarray(Wv, dtype=np.float32)
    Wo = np.asarray(Wo, dtype=np.float32)
    bq = np.asarray(bq, dtype=np.float32)
    bk = np.asarray(bk, dtype=np.float32)
    bv = np.asarray(bv, dtype=np.float32)
    bo = np.asarray(bo, dtype=np.float32)

    nc = _get_nc()
    maps = make_in_maps(q, k, v, Wq, bq, Wk, bk, Wv, bv, Wo, bo)
    res = run_bass_kernel_spmd(nc, maps, core_ids=list(range(8)))

    outs = []
    for b in range(B):
        acc = np.zeros((S, D), dtype=np.float32)
        for hc in range(4):
            part = res.results[b * 4 + hc]["out"]  # [2, S, D] bf16
            acc += part[0].astype(np.float32)
            acc += part[1].astype(np.float32)
        acc += bo[None, :]
        outs.append(acc)
    return np.stack(outs, axis=0)


# revision 16
# speedup vs baseline: 1.1947x; 1.1947x over previous
"""Bass/Tile kernel for nn_MultiHeadAttention (B=2, S=2048, D=1024, H=16).

Sharding: 8 cores = 2 (batch) x 4 (head-chunks of 4 heads).
Each core computes, for its batch b and its 4 heads (2 pairs of 2 heads):
  qpT/kpT = (x @ W{q,k} + b)^T   in [dout, token] bf16 layout
  vp      = v @ Wv + bv          in [token, dout] bf16 layout
  scoresT = kp @ qp^T            per head, [k, q] f32 PSUM
  at      = exp(scoresT) bf16, Z[k] via ACT accum_out
  pv[qh]  = sum_kb (vp/Z)^T @ at  accumulated IN PSUM across all kb
  out[p]  = hcT_p^T @ Wo_p        per-pair bf16 partial (host sums 8 partials)

All matmuls bf16 (rel err ~1.25e-2 vs 2e-2 gate, matches numpy sim exactly).
v3: resident raw q/k/v tiles loaded with 2KB-per-partition DMA lines (2x DMA
efficiency vs 512-token chunks), packed wqkv weight tensor, early-exp pass
(scores for kb 0-3 on the first q-half start right after q-proj chunks 0/1 +
k-proj chunk 0), PV(kb-1) emitted between score groups to keep the PE stream
dense (DVFS ramp needs continuous tensor work), per-pair O-proj overlapped
with the other pair's attention.
"""

import sys

sys.path.insert(0, "/opt/trn_rl_repo")

from contextlib import ExitStack

import numpy as np
import ml_dtypes

import concourse.bass as bass
import concourse.mybir as mybir
import concourse.tile as tile
from concourse import bacc
from concourse.bass_utils import run_bass_kernel_spmd

BF16 = mybir.dt.bfloat16
F32 = mybir.dt.float32
AF = mybir.ActivationFunctionType
ALU = mybir.AluOpType

D = 1024
NK = 8  # k-tiles over D
DOUT = 256  # per-core head dims (4 heads)
NPAIR = 2  # pairs of heads (128 dout each)
HD = 64
S = 2048
B = 2
NKB = S // 128  # k-token blocks
NQH = S // 1024  # 1024-wide q halves
NTC = S // 512  # proj token chunks
NTT = S // 128  # token tiles
EARLY = 4  # kb blocks whose qh=0 scores/exp run during the projection head


def build_kernel():
    nc = bacc.Bacc("TRN2", target_bir_lowering=False, debug=False)

    qT = nc.dram_tensor("qT", [D, S], BF16, kind="ExternalInput")
    kT = nc.dram_tensor("kT", [D, S], BF16, kind="ExternalInput")
    vT = nc.dram_tensor("vT", [D, S], BF16, kind="ExternalInput")
    wqkv = nc.dram_tensor("wqkv", [D, 3 * DOUT], BF16, kind="ExternalInput")
    wo = nc.dram_tensor("wo", [DOUT, D], BF16, kind="ExternalInput")
    bq = nc.dram_tensor("bq", [NPAIR, 128, 1], F32, kind="ExternalInput")
    bk = nc.dram_tensor("bk", [NPAIR, 128, 1], F32, kind="ExternalInput")
    bv = nc.dram_tensor("bv", [DOUT], F32, kind="ExternalInput")
    out = nc.dram_tensor("out", [NPAIR, S, D], BF16, kind="ExternalOutput")

    qTv = qT.ap().rearrange("(t p) s -> t p s", p=128)  # [8, 128, S]
    kTv = kT.ap().rearrange("(t p) s -> t p s", p=128)
    vTv = vT.ap().rearrange("(t p) s -> t p s", p=128)
    wqkvv = wqkv.ap().rearrange("(t p) m -> t p m", p=128)  # [8, 128, 768]
    wov = wo.ap().rearrange("(t p) m -> t p m", p=128)  # [2, 128, 1024]
    bqv = bq.ap().rearrange("a p o -> p a o")  # [128, 2, 1]
    bkv = bk.ap().rearrange("a p o -> p a o")
    outv = out.ap().rearrange("a (t p) m -> a t p m", p=128)  # [2,16,128,1024]

    bv_bcast_ap = bass.AP(tensor=bv.ap().tensor, offset=0, ap=[[0, 128], [1, DOUT]])

    with tile.TileContext(nc) as tc, ExitStack() as ctx:
        sb = ctx.enter_context(tc.tile_pool(name="sb", bufs=1))

        # resident raw inputs + weights
        qT_sb = sb.tile([128, NK, S], BF16, tag="qT")
        kT_sb = sb.tile([128, NK, S], BF16, tag="kT")
        vT_sb = sb.tile([128, NK, S], BF16, tag="vT")
        wqkv_sb = sb.tile([128, NK, 3 * DOUT], BF16, tag="wqkv")
        wo_sb = sb.tile([128, NPAIR, D], BF16, tag="wo")
        bq_sb = sb.tile([128, NPAIR, 1], F32, tag="bq")
        bk_sb = sb.tile([128, NPAIR, 1], F32, tag="bk")
        bv_sb = sb.tile([128, DOUT], F32, tag="bv")

        # projection outputs (resident)
        qpT_sb = sb.tile([128, NPAIR, S], BF16, tag="qpT")
        kpT_sb = sb.tile([128, NPAIR, S], BF16, tag="kpT")
        vp_sb = sb.tile([128, NTT, DOUT], BF16, tag="vp")
        hcT_sb = sb.tile([128, NPAIR, S], BF16, tag="hcT")
        hc_acc = sb.tile([128, S], F32, tag="hc_acc")  # per-pair, reused

        psa = ctx.enter_context(tc.tile_pool(name="ps_all", bufs=1, space="PSUM"))
        asb = ctx.enter_context(tc.tile_pool(name="att_sb", bufs=1))
        osb = ctx.enter_context(tc.tile_pool(name="o_sb", bufs=1))

        # ---- DMA schedule: ordered by first consumption ----
        # wqkv[0] + qA first so the very first matmul can start early.
        nc.sync.dma_start(out=wqkv_sb[:, 0, :], in_=wqkvv[0])
        nc.sync.dma_start(out=bq_sb[:], in_=bqv)
        for kk in range(NK):  # q tokens 0..1023
            nc.sync.dma_start(out=qT_sb[:, kk, 0:1024], in_=qTv[kk][:, 0:1024])
        for kk in range(1, NK):
            nc.sync.dma_start(out=wqkv_sb[:, kk, :], in_=wqkvv[kk])
        nc.sync.dma_start(out=bk_sb[:], in_=bkv)
        nc.sync.dma_start(out=bv_sb[:], in_=bv_bcast_ap)
        for kk in range(NK):  # k tokens 0..1023
            nc.sync.dma_start(out=kT_sb[:, kk, 0:1024], in_=kTv[kk][:, 0:1024])
        for kk in range(NK):  # q tokens 1024..2047
            nc.sync.dma_start(out=qT_sb[:, kk, 1024:2048], in_=qTv[kk][:, 1024:2048])
        for kk in range(NK):  # v tokens 0..1023
            nc.sync.dma_start(out=vT_sb[:, kk, 0:1024], in_=vTv[kk][:, 0:1024])
        for kk in range(NK):  # k tokens 1024..2047
            nc.sync.dma_start(out=kT_sb[:, kk, 1024:2048], in_=kTv[kk][:, 1024:2048])
        for kk in range(NK):  # v tokens 1024..2047
            nc.sync.dma_start(out=vT_sb[:, kk, 1024:2048], in_=vTv[kk][:, 1024:2048])
        for t in range(NPAIR):
            nc.sync.dma_start(out=wo_sb[:, t, :], in_=wov[t])

        # ---- emission helpers ----
        BIGB = 4  # 'big' PSUM slots: 4 x [128,1024] f32 = all 8 banks

        def emit_qkproj_pair(XT_sb, woff, b_sb, XPT, tci, p):
            tsl = slice(tci * 512, tci * 512 + 512)
            ps_t = psa.tile([128, 512], F32, tag="big", bufs=BIGB, name=f"pj{p}")
            for kk in range(NK):
                nc.tensor.matmul(
                    ps_t[:],
                    lhsT=wqkv_sb[:, kk, woff + p * 128 : woff + p * 128 + 128],
                    rhs=XT_sb[:, kk, tsl],
                    start=(kk == 0),
                    stop=(kk == NK - 1),
                )
            nc.vector.tensor_scalar_add(XPT[:, p, tsl], ps_t[:], b_sb[:, p, :])

        def emit_vproj(tt):
            psv = psa.tile([128, DOUT], F32, tag="big", bufs=BIGB, name="projv")
            for kk in range(NK):
                nc.tensor.matmul(
                    psv[:],
                    lhsT=vT_sb[:, kk, tt * 128 : tt * 128 + 128],
                    rhs=wqkv_sb[:, kk, 2 * DOUT : 3 * DOUT],
                    start=(kk == 0),
                    stop=(kk == NK - 1),
                )
            nc.vector.scalar_tensor_tensor(
                out=vp_sb[:, tt, :],
                in0=psv[:],
                scalar=1.0,
                in1=bv_sb[:],
                op0=ALU.mult,
                op1=ALU.add,
            )

        def emit_oproj(p, tt, tail=False):
            ost = osb.tile([128, D], BF16, tag="ost", bufs=3, name="ost")
            pso = psa.tile([128, D], F32, tag="big", bufs=BIGB, name="o")
            for dc in range(2):
                nc.tensor.matmul(
                    pso[:, dc * 512 : dc * 512 + 512],
                    lhsT=hcT_sb[:, p, tt * 128 : tt * 128 + 128],
                    rhs=wo_sb[:, p, dc * 512 : dc * 512 + 512],
                    start=True,
                    stop=True,
                )
            if tail:
                nc.scalar.copy(ost[:], pso[:])
            else:
                nc.vector.tensor_copy(ost[:], pso[:])
            nc.sync.dma_start(out=outv[p][tt], in_=ost[:])

        def emit_sc_exp(p, kb, h, qh, z4):
            ksl = slice(kb * 128, kb * 128 + 128)
            hsl = slice(h * 64, h * 64 + 64)
            sc = psa.tile([128, 1024], F32, tag="big", bufs=BIGB, name=f"sc{h}{qh}")
            for qq in range(2):
                qsl = slice(qh * 1024 + qq * 512, qh * 1024 + qq * 512 + 512)
                nc.tensor.matmul(
                    sc[:, qq * 512 : qq * 512 + 512],
                    lhsT=kpT_sb[hsl, p, ksl],
                    rhs=qpT_sb[hsl, p, qsl],
                    start=True,
                    stop=True,
                )
            at = asb.tile([128, 1024], BF16, tag="at", bufs=12, name=f"at{h}{qh}")
            nc.scalar.activation(
                out=at[:], in_=sc[:], func=AF.Exp,
                accum_out=z4[:, h, qh : qh + 1],
            )
            return at

        def emit_zchain(p, kb, z4):
            z2 = asb.tile([128, 2], F32, tag="z2", bufs=4, name="z2")
            nc.vector.tensor_add(z2[:], z4[:, :, 0], z4[:, :, 1])
            rz = asb.tile([128, 2], F32, tag="rz", bufs=4, name="rz")
            nc.vector.reciprocal(rz[:], z2[:])
            vhs = asb.tile([128, 2, HD], BF16, tag="vhs", bufs=6, name="vhs")
            for h in range(2):
                nc.vector.tensor_scalar_mul(
                    vhs[:, h, :],
                    vp_sb[:, kb, p * 128 + h * 64 : p * 128 + h * 64 + 64],
                    rz[:, h : h + 1],
                )
            return vhs

        def emit_pv(kb, ats, vhs):
            for qh in range(NQH):
                pvt = psa.tile([128, 1024], F32, tag="big", bufs=BIGB, name="pvt")
                for h in range(2):
                    for qq in range(2):
                        nc.tensor.matmul(
                            pvt[h * 64 : h * 64 + 64, qq * 512 : qq * 512 + 512],
                            lhsT=vhs[:, h, :],
                            rhs=ats[(h, qh)][:, qq * 512 : qq * 512 + 512],
                            start=True,
                            stop=True,
                            tile_position=(0, h * 64),
                            skip_group_check=True,
                        )
                qsl = slice(qh * 1024, qh * 1024 + 1024)
                if kb == 0:
                    nc.vector.tensor_copy(hc_acc[:, qsl], pvt[:])
                else:
                    nc.vector.tensor_add(hc_acc[:, qsl], hc_acc[:, qsl], pvt[:])

        # ---- projection head + early exp pass (pair 0, kb<EARLY, qh=0) ----
        for tci in (0, 1):
            for p in range(NPAIR):
                emit_qkproj_pair(qT_sb, 0, bq_sb, qpT_sb, tci, p)
        for p in range(NPAIR):
            emit_qkproj_pair(kT_sb, DOUT, bk_sb, kpT_sb, 0, p)

        at_early = {}
        z4_early = {}
        for kb in range(EARLY):
            z4 = asb.tile([128, 2, NQH], F32, tag="z4e", bufs=EARLY + 1, name="z4e")
            z4_early[kb] = z4
            for h in range(2):
                at_early[(kb, h)] = emit_sc_exp(0, kb, h, 0, z4)
            # q-proj chunks 2/3 ride the early pass's PE slack
            emit_qkproj_pair(qT_sb, 0, bq_sb, qpT_sb, 2 + kb // 2, kb % 2)
        emit_vproj(0)
        emit_vproj(1)

        # ---- attention main loops ----
        for p in range(NPAIR):
            prev = None
            for kb in range(NKB):
                if p == 0 and kb < EARLY:
                    z4 = z4_early[kb]
                    ats = {
                        (0, 0): at_early[(kb, 0)],
                        (1, 0): at_early[(kb, 1)],
                    }
                    ats[(0, 1)] = emit_sc_exp(p, kb, 0, 1, z4)
                    if prev is not None:
                        emit_pv(kb - 1, prev[0], prev[1])
                    ats[(1, 1)] = emit_sc_exp(p, kb, 1, 1, z4)
                else:
                    z4 = asb.tile(
                        [128, 2, NQH], F32, tag="z4", bufs=6, name="z4"
                    )
                    ats = {}
                    ats[(0, 0)] = emit_sc_exp(p, kb, 0, 0, z4)
                    ats[(0, 1)] = emit_sc_exp(p, kb, 0, 1, z4)
                    if prev is not None:
                        emit_pv(kb - 1, prev[0], prev[1])
                    ats[(1, 0)] = emit_sc_exp(p, kb, 1, 0, z4)
                    ats[(1, 1)] = emit_sc_exp(p, kb, 1, 1, z4)
                vhs = emit_zchain(p, kb, z4)
                prev = (ats, vhs)
                # trickled, dependency-free PE filler
                if p == 0:
                    if kb < 14:
                        emit_vproj(kb + 2)
                    if kb in (0, 1):
                        emit_qkproj_pair(kT_sb, DOUT, bk_sb, kpT_sb, 1, kb)
                    if kb in (5, 6):
                        emit_qkproj_pair(
                            kT_sb, DOUT, bk_sb, kpT_sb, 2, kb - 5
                        )
                    if kb in (9, 10):
                        emit_qkproj_pair(
                            kT_sb, DOUT, bk_sb, kpT_sb, 3, kb - 9
                        )
                else:
                    emit_oproj(0, kb)
            emit_pv(NKB - 1, prev[0], prev[1])
            for qh in range(NQH):
                qsl = slice(qh * 1024, qh * 1024 + 1024)
                nc.vector.tensor_copy(hcT_sb[:, p, qsl], hc_acc[:, qsl])

        # ---- pair-1 O-projection tail ----
        for tt in range(NTT):
            emit_oproj(1, tt, tail=True)

    nc.compile()
    return nc


# ---------------- host-side shard / unshard ----------------

_NC_CACHE = {}


def _get_nc():
    if "nc" not in _NC_CACHE:
        _NC_CACHE["nc"] = build_kernel()
    return _NC_CACHE["nc"]


def make_in_maps(q, k, v, Wq, bq, Wk, bk, Wv, bv, Wo, bo):
    bf = ml_dtypes.bfloat16
    maps = []
    qb = [np.ascontiguousarray(q[b].T.astype(bf)) for b in range(B)]
    kb_ = [np.ascontiguousarray(k[b].T.astype(bf)) for b in range(B)]
    vb = [np.ascontiguousarray(v[b].T.astype(bf)) for b in range(B)]
    for c in range(8):
        b = c // 4
        hc = c % 4
        cols = slice(256 * hc, 256 * hc + 256)
        wqkv = np.concatenate(
            [Wq[:, cols], Wk[:, cols], Wv[:, cols]], axis=1
        ).astype(bf)
        maps.append({
            "qT": qb[b],
            "kT": kb_[b],
            "vT": vb[b],
            "wqkv": np.ascontiguousarray(wqkv),
            "wo": np.ascontiguousarray(Wo[cols, :].astype(bf)),
            "bq": np.ascontiguousarray(
                bq[cols].reshape(NPAIR, 128, 1).astype(np.float32)
            ),
            "bk": np.ascontiguousarray(
                bk[cols].reshape(NPAIR, 128, 1).astype(np.float32)
            ),
            "bv": np.ascontiguousarray(bv[cols].astype(np.float32)),
        })
    return maps


def kernel(q, k, v, Wq, bq, Wk, bk, Wv, bv, Wo, bo):
    q = np.asarray(q, dtype=np.float32)
    k = np.asarray(k, dtype=np.float32)
    v = np.asarray(v, dtype=np.float32)
    Wq = np.asarray(Wq, dtype=np.float32)
    Wk = np.asarray(Wk, dtype=np.float32)
    Wv = np.asarray(Wv, dtype=np.float32)
    Wo = np.asarray(Wo, dtype=np.float32)
    bq = np.asarray(bq, dtype=np.float32)
    bk = np.asarray(bk, dtype=np.float32)
    bv = np.asarray(bv, dtype=np.float32)
    bo = np.asarray(bo, dtype=np.float32)

    nc = _get_nc()
    maps = make_in_maps(q, k, v, Wq, bq, Wk, bk, Wv, bv, Wo, bo)
    res = run_bass_kernel_spmd(nc, maps, core_ids=list(range(8)))

    outs = []
    for b in range(B):
        acc = np.zeros((S, D), dtype=np.float32)
        for hc in range(4):
            part = res.results[b * 4 + hc]["out"]  # [2, S, D] bf16
            acc += part[0].astype(np.float32)
            acc += part[1].astype(np.float32)
        acc += bo[None, :]
        outs.append(acc)
    return np.stack(outs, axis=0)
